# revision 18
# baseline (speedup 1.0000x reference)
"""Trainium2 Bass kernel for nn_DecoderBlock (dense transformer decoder block).

Sharding: data-parallel over batch (8 batch elements -> 8 NeuronCores), no
collectives. Each core computes one full decoder block on [S=1024, D=1024].

Per-core strategy (v4):
  - activations feature-major ([D, S]); weights+activations bf16, fp32 PSUM
  - weights pre-swizzled on host to [128, kt, n] so every weight load is one
    DMA with 16KB-contiguous per-partition lines (descriptor-efficient)
  - attention computed in phases: Q-all, K-all, V-all (N=512), then 16 heads
    as 8 packed pairs (row groups 0/64 run concurrently in the PE array)
  - softmax without max-subtraction; causal mask via precomputed 0/1 bf16
    mask tiles on DVE; V carries a ones column so denominators fall out of
    the attn@v matmul
  - LN1 on fp32 input chunks (natural layout, bn_stats) fused into the input
    transpose; LN2/LN3 feature-major via ones-matmul stats
"""
import sys

sys.path.insert(0, '/opt/trn_rl_repo')

import contextlib

import numpy as np
import ml_dtypes

import concourse.bacc as bacc
import concourse.mybir as mybir
import concourse.tile as tile
from concourse.bass_utils import run_bass_kernel_spmd
from concourse.masks import make_identity

f32 = mybir.dt.float32
f32r = mybir.dt.float32r
bf16 = mybir.dt.bfloat16
AF = mybir.ActivationFunctionType
ALU = mybir.AluOpType

B, S, D, H, HD, FF = 8, 1024, 1024, 16, 64, 4096
ST = S // 128   # 8
DT = D // 128   # 8
FT = FF // 128  # 32
EPS = 1e-5
ISQ = 1.0 / 8.0  # 1/sqrt(HD)

W_NAMES = ['sa_wq', 'sa_wk', 'sa_wv', 'sa_wo', 'ca_wq', 'ca_wk', 'ca_wv', 'ca_wo']
B_NAMES = ['sa_bq', 'sa_bk', 'sa_bv', 'sa_bo', 'ca_bq', 'ca_bk', 'ca_bv', 'ca_bo']
LN_NAMES = ['ln1_g', 'ln1_b', 'ln2_g', 'ln2_b', 'ln3_g', 'ln3_b']


def _build(iters=1):
    nc = bacc.Bacc("TRN2", target_bir_lowering=False, debug=False, num_devices=8)

    dec_d = nc.dram_tensor("decoder", [S, D], f32, kind="ExternalInput").ap()
    enc_d = nc.dram_tensor("encoder", [S, D], f32, kind="ExternalInput").ap()
    # weights pre-swizzled on host (see _in_maps)
    wd = {n: nc.dram_tensor(n, [128, DT, D], bf16, kind="ExternalInput").ap()
          for n in W_NAMES}
    bd = {n: nc.dram_tensor(n, [1, D] if n.endswith('bv') else [D], f32,
                            kind="ExternalInput").ap() for n in B_NAMES}
    lnd = {n: nc.dram_tensor(n, [D], f32, kind="ExternalInput").ap() for n in LN_NAMES}
    w1_d = nc.dram_tensor("ffn_w1", [128, 4, DT, D], bf16, kind="ExternalInput").ap()
    b1_d = nc.dram_tensor("ffn_b1", [FF], f32, kind="ExternalInput").ap()
    w2_d = nc.dram_tensor("ffn_w2", [128, 4, DT, D], bf16, kind="ExternalInput").ap()
    b2_d = nc.dram_tensor("ffn_b2", [D], f32, kind="ExternalInput").ap()
    out_d = nc.dram_tensor("out", [S, D], f32, kind="ExternalOutput").ap()

    with tile.TileContext(nc) as tc, \
            nc.allow_low_precision(reason="bf16 matmul pipeline by design"):
        _body(nc, tc, dec_d, enc_d, wd, bd, lnd, w1_d, b1_d, w2_d, b2_d, out_d, iters)
    nc.compile()
    return nc


def _body(nc, tc, dec_d, enc_d, wd, bd, lnd, w1_d, b1_d, w2_d, b2_d, out_d, iters):
    ctx = contextlib.ExitStack()
    with ctx:
        persist = ctx.enter_context(tc.tile_pool(name="persist", bufs=1))
        big = ctx.enter_context(tc.tile_pool(name="big", bufs=1))
        un = ctx.enter_context(tc.tile_pool(name="un", bufs=3))
        chk = ctx.enter_context(tc.tile_pool(name="chk", bufs=3))
        att = ctx.enter_context(tc.tile_pool(name="att", bufs=4))
        wp = ctx.enter_context(tc.tile_pool(name="wp", bufs=2))
        sm = ctx.enter_context(tc.tile_pool(name="sm", bufs=2))
        ps_a = ctx.enter_context(tc.tile_pool(name="ps_a", bufs=2, space="PSUM"))
        ps_r = ctx.enter_context(tc.tile_pool(name="ps_r", bufs=4, space="PSUM"))

        # ---- persistent constants ----
        ones_f = persist.tile([128, 16], f32, tag="ones_f")
        nc.vector.memset(ones_f, 1.0)
        ones_col = persist.tile([128, 1], bf16, tag="ones_col")
        nc.vector.tensor_copy(ones_col, ones_f[:, 0:1])
        onesr_f = persist.tile([1, 128], f32, tag="onesr_f")
        nc.vector.memset(onesr_f, 1.0)
        ones_row = persist.tile([1, 128], f32r, tag="ones_row")
        nc.vector.tensor_copy(ones_row, onesr_f)
        ident = persist.tile([128, 128], f32, tag="ident")
        make_identity(nc, ident)
        ident_b = persist.tile([128, 128], bf16, tag="ident_b")
        nc.vector.tensor_copy(ident_b, ident)
        eps1 = persist.tile([1, 1], f32, tag="eps1")
        nc.vector.memset(eps1, EPS)
        eps128 = persist.tile([128, 1], f32, tag="eps128")
        nc.vector.memset(eps128, EPS)

        bias_t = {}
        for n in ['sa_bq', 'sa_bk', 'sa_bo', 'ca_bq', 'ca_bk', 'ca_bo']:
            bias_t[n] = persist.tile([128, DT], f32, tag=n, name=n)
            nc.sync.dma_start(bias_t[n], bd[n].rearrange("(t p) -> p t", p=128))
        for n in LN_NAMES:
            bias_t[n] = persist.tile([128, DT], f32, tag=n, name=n)
            nc.sync.dma_start(bias_t[n], lnd[n].rearrange("(t p) -> p t", p=128))
        bias_t['ffn_b1'] = persist.tile([128, FT], f32, tag="ffn_b1", name="ffn_b1")
        nc.sync.dma_start(bias_t['ffn_b1'], b1_d.rearrange("(t p) -> p t", p=128))
        bias_t['ffn_b2'] = persist.tile([128, DT], f32, tag="ffn_b2", name="ffn_b2")
        nc.sync.dma_start(bias_t['ffn_b2'], b2_d.rearrange("(t p) -> p t", p=128))
        bvfull = persist.tile([1, D], f32r, tag="bvfull")

        # causal 0/1 masks: mask4[:, i, h, j] = 1 if j >= p + 128*i else 0,
        # duplicated across h=2 so one DVE multiply masks a packed head pair
        mask4 = persist.tile([128, 4, 2, 512], bf16, tag="mask4")
        for i in range(4):
            mtmp = sm.tile([128, 512], f32r, tag="sq", name="mtmp")
            nc.vector.memset(mtmp.bitcast(f32), 1.0)
            nc.gpsimd.affine_select(
                out=mtmp, in_=mtmp, compare_op=ALU.is_ge, fill=0.0,
                base=-128 * i, pattern=[[1, 512]], channel_multiplier=-1)
            for hh in range(2):
                nc.vector.tensor_copy(mask4[:, i, hh, :], mtmp)

        # stats scratch rows (single-buffered, reused per LN/softmax call)
        a_row = persist.tile([1, S], f32r, tag="a_row")
        c_row = persist.tile([1, S], f32r, tag="c_row")
        rowA = persist.tile([1, 512], f32, tag="rowA")
        rowB = persist.tile([1, 512], f32, tag="rowB")
        rowC = persist.tile([1, 512], f32, tag="rowC")
        rec = persist.tile([1, 512], f32r, tag="rec")

        # ---- big bf16 activation buffers (2 MB each) ----
        bufA = big.tile([128, DT, S], bf16, tag="bufA")
        bufB = big.tile([128, DT, S], bf16, tag="bufB")
        bufC = big.tile([128, DT, S], bf16, tag="bufC")
        repB = big.tile([128, DT, S], bf16, tag="repB")

        def mm(out_ap, lhsT_ap, rhs_ap, start, stop):
            nc.tensor.matmul(out_ap, lhsT_ap, rhs_ap, start=start, stop=stop,
                             skip_group_check=True)

        def load_w(w_view):
            t = wp.tile([128, DT, D], bf16, tag="w")
            nc.sync.dma_start(t, w_view)
            return t

        def proj_T(w_dram, bias, src_T, dst_T, residual=None):
            wt = load_w(w_dram)
            for m in range(DT):
                ps = ps_a.tile([128, S], f32, tag="a")
                for k in range(DT):
                    for c in range(2):
                        cs = slice(c * 512, (c + 1) * 512)
                        mm(ps[:, cs], wt[:, k, m * 128:(m + 1) * 128],
                           src_T[:, k, cs], k == 0, k == DT - 1)
                if residual is None:
                    nc.vector.tensor_scalar(
                        dst_T[:, m, :], ps, bias[:, m:m + 1], None, ALU.add)
                else:
                    nc.vector.scalar_tensor_tensor(
                        dst_T[:, m, :], ps, bias[:, m:m + 1],
                        residual[:, m, :], ALU.add, ALU.add)

        def stage_in(src_d, dst_T, ln1=False):
            # natural fp32 rows -> feature-major bf16 dst, optional LN1 fused
            for st in range(ST):
                ch = chk.tile([128, D], f32, tag="chk")
                nc.sync.dma_start(ch.bitcast(f32r),
                                  src_d[st * 128:(st + 1) * 128, :].bitcast(f32r))
                if ln1:
                    stats = sm.tile([128, 2, 6], f32, tag="bnst")
                    xr = ch.rearrange("p (g d) -> p g d", g=2)
                    for g2 in range(2):
                        nc.vector.bn_stats(stats[:, g2, :], xr[:, g2, :])
                    mv = sm.tile([128, 2], f32, tag="bnmv")
                    nc.vector.bn_aggr(mv, stats)
                    std = sm.tile([128, 1], f32, tag="bnstd")
                    nc.scalar.activation(std, mv[:, 1:2], AF.Sqrt, bias=eps128)
                    rstd = sm.tile([128, 1], f32, tag="bnrstd")
                    nc.vector.reciprocal(rstd, std)
                    nc.vector.tensor_scalar(ch.bitcast(f32r), ch,
                                            mv[:, 0:1], rstd,
                                            ALU.subtract, ALU.mult)
                for j in range(DT):
                    tp = ps_r.tile([128, 512], f32, tag="r")
                    nc.tensor.transpose(tp[:, 0:128], ch[:, j * 128:(j + 1) * 128],
                                        ident)
                    dst = dst_T[:, j, st * 128:(st + 1) * 128]
                    if ln1:
                        nc.vector.tensor_scalar(
                            dst, tp[:, 0:128],
                            bias_t['ln1_g'][:, j:j + 1], bias_t['ln1_b'][:, j:j + 1],
                            ALU.mult, ALU.add)
                    else:
                        nc.vector.tensor_copy(dst, tp[:, 0:128])

        def ln_partition(T, g_ap, b_ap):
            # in-place layernorm over the feature (partition-tiled) dim of T
            sums = [ps_r.tile([1, 512], f32, tag="r", name=f"sums{_c}") for _c in range(2)]
            ssqs = [ps_r.tile([1, 512], f32, tag="r", name=f"ssqs{_c}") for _c in range(2)]
            for t in range(DT):
                for c in range(2):
                    cs = slice(c * 512, (c + 1) * 512)
                    sq = sm.tile([128, 512], bf16, tag="sqb")
                    nc.vector.tensor_mul(sq, T[:, t, cs], T[:, t, cs])
                    mm(sums[c], ones_col, T[:, t, cs], t == 0, t == DT - 1)
                    mm(ssqs[c], ones_col, sq, t == 0, t == DT - 1)
            for c in range(2):
                cs = slice(c * 512, (c + 1) * 512)
                nc.vector.tensor_scalar(rowA, sums[c], 1.0 / D, None, ALU.mult)  # mu
                nc.vector.tensor_scalar(rowB, ssqs[c], 1.0 / D, None, ALU.mult)  # E[x^2]
                nc.vector.scalar_tensor_tensor(rowC, rowA, -1.0, rowA, ALU.mult,
                                               ALU.mult)                          # -mu^2
                nc.vector.tensor_add(rowB, rowB, rowC)                            # var
                nc.scalar.activation(rowC, rowB, AF.Sqrt, bias=eps1)              # std
                nc.vector.reciprocal(rowB, rowC)                                  # rstd
                nc.vector.tensor_copy(a_row[:, cs], rowB)
                nc.vector.scalar_tensor_tensor(c_row[:, cs], rowA, -1.0, rowB,
                                               ALU.mult, ALU.mult)                # -mu*rstd
            bcA = ps_a.tile([128, S], f32, tag="a")
            bcC = ps_a.tile([128, S], f32, tag="a")
            for c in range(2):
                cs = slice(c * 512, (c + 1) * 512)
                mm(bcA[:, cs], ones_row, a_row[:, cs], True, True)
                mm(bcC[:, cs], ones_row, c_row[:, cs], True, True)
            for t in range(DT):
                for c in range(2):
                    cs = slice(c * 512, (c + 1) * 512)
                    tmp = sm.tile([128, 512], f32r, tag="sq", name="lntmp")
                    nc.vector.tensor_scalar(tmp, bcC[:, cs], g_ap[:, t:t + 1],
                                            b_ap[:, t:t + 1], ALU.mult, ALU.add)
                    nc.vector.tensor_mul(T[:, t, cs], T[:, t, cs], bcA[:, cs])
                    nc.vector.scalar_tensor_tensor(
                        T[:, t, cs], T[:, t, cs], g_ap[:, t:t + 1],
                        tmp, ALU.mult, ALU.add)

        def attention(src_q, src_kv, pre, causal, dst_T, residual_T, repT):
            wq, wk, wv, wo = (wd[pre + n] for n in ('wq', 'wk', 'wv', 'wo'))
            bq, bk, bo = (bias_t[pre + n] for n in ('bq', 'bk', 'bo'))
            bd_bv = bd[pre + 'bv']

            qA = un.tile([128, DT, S], bf16, tag="u", name="qA")
            kA = un.tile([128, DT, S], bf16, tag="u", name="kA")
            vA = un.tile([128, ST, H, 65], bf16, tag="u", name="vA")

            for wmat, bmat, dstq, srcx in ((wq, bq, qA, src_q), (wk, bk, kA, src_kv)):
                wt = load_w(wmat)
                for m in range(DT):
                    ps = ps_a.tile([128, S], f32, tag="a")
                    for k in range(DT):
                        for c in range(2):
                            cs = slice(c * 512, (c + 1) * 512)
                            mm(ps[:, cs], wt[:, k, m * 128:(m + 1) * 128],
                               srcx[:, k, cs], k == 0, k == DT - 1)
                    nc.vector.tensor_scalar(
                        dstq[:, m, :], ps, bmat[:, m:m + 1], None, ALU.add)

            # V in natural layout: vA[s, head, hd] (+bias, +ones column)
            wt = load_w(wv)
            nc.sync.dma_start(bvfull, bd_bv.bitcast(f32r))
            bcv = ps_a.tile([128, S], f32, tag="a")
            for c in range(2):
                cs = slice(c * 512, (c + 1) * 512)
                mm(bcv[:, cs], ones_row, bvfull[:, cs], True, True)
            bcv_sb = sm.tile([128, D], bf16, tag="bcv", name="bcv_sb")
            nc.vector.tensor_copy(bcv_sb, bcv)
            for st in range(ST):
                psv = ps_a.tile([128, S], f32, tag="a")
                for half in range(2):
                    hs = slice(half * 512, (half + 1) * 512)
                    for k in range(DT):
                        mm(psv[:, hs], src_kv[:, k, st * 128:(st + 1) * 128],
                           wt[:, k, hs], k == 0, k == DT - 1)
                for half in range(2):
                    hs = slice(half * 512, (half + 1) * 512)
                    nc.vector.tensor_add(
                        vA[:, st, half * 8:(half + 1) * 8, 0:64],
                        psv[:, hs].rearrange("p (h e) -> p h e", h=8),
                        bcv_sb[:, hs].rearrange("p (h e) -> p h e", h=8))
                nc.vector.tensor_copy(vA[:, st, :, 64:65], ones_f.unsqueeze(2))

            # 16 heads as 8 packed pairs (row groups 0 / 64)
            for m in range(DT):
                for c in range(2):
                    cs = slice(c * 512, (c + 1) * 512)
                    contrib = [skt for skt in range(ST)
                               if not (causal and skt * 128 > c * 512 + 511)]
                    rp = ps_a.tile([128, S], f32, tag="a")
                    for idx, skt in enumerate(contrib):
                        ss = slice(skt * 128, (skt + 1) * 128)
                        at2 = att.tile([128, 2, 512], bf16, tag="at", name="at2")
                        scs = []
                        for hh in range(2):
                            sc = ps_r.tile([128, 512], f32, tag="r", name=f"sc{hh}")
                            mm(sc, kA[hh * 64:hh * 64 + 64, m, ss],
                               qA[hh * 64:hh * 64 + 64, m, cs], True, True)
                            scs.append(sc)
                        doff = skt * 128 - c * 512
                        for hh in range(2):
                            nc.scalar.activation(at2[:, hh, :], scs[hh], AF.Exp,
                                                 scale=ISQ)
                        if causal and doff >= 0:
                            nc.vector.tensor_mul(at2, at2,
                                                 mask4[:, doff // 128, :, :])
                        for hh in range(2):
                            mm(rp[0:65, hh * 512:(hh + 1) * 512],
                               vA[:, skt, 2 * m + hh, 0:65], at2[:, hh, :],
                               idx == 0, idx == len(contrib) - 1)
                    for hh in range(2):
                        rp_h = rp[:, hh * 512:(hh + 1) * 512]
                        nc.vector.reciprocal(rec, rp_h[64:65, :])
                        bcr = ps_r.tile([128, 512], f32, tag="r")
                        mm(bcr[0:64, :], ones_row[:, 0:64], rec, True, True)
                        bcr_sb = sm.tile([64, 512], f32r, tag="sq", name="bcr_sb")
                        nc.vector.tensor_copy(bcr_sb, bcr[0:64, :])
                        nc.vector.tensor_mul(
                            repT[hh * 64:hh * 64 + 64, m, cs],
                            rp_h[0:64, :], bcr_sb)
            proj_T(wo, bo, repT, dst_T, residual=residual_T)

        # ================= block body =================
        def block_body(_i=None):
            # P1: decoder + LN1 -> xT (bufA)
            stage_in(dec_d, bufA, ln1=True)

            # P3: self-attention (causal), residual xT -> x2T (bufB)
            attention(bufA, bufA, 'sa_', True, bufB, bufA, repB)

            # P4.5: encoder natural -> encT (bufA); overlaps with LN2 below
            stage_in(enc_d, bufA)

            # P4: LN2 in-place -> yT
            ln_partition(bufB, bias_t['ln2_g'], bias_t['ln2_b'])

            # P5: cross-attention, residual yT -> y2T (bufC)
            attention(bufB, bufA, 'ca_', False, bufC, bufB, repB)

            # P6: LN3 in-place -> zT
            ln_partition(bufC, bias_t['ln3_g'], bias_t['ln3_b'])

            # P7: FFN in two FF halves (h tiles in union slots)
            for hf in range(2):
                hbufs = [un.tile([128, DT, S], bf16, tag="u", name=f"h{hf}_{_b}")
                         for _b in range(2)]
                for q4 in range(2):
                    w1t = load_w(w1_d[:, hf * 2 + q4, :, :])
                    for lm in range(DT):
                        ft = (hf * 2 + q4) * 8 + lm
                        mi = q4 * 8 + lm
                        ps = ps_a.tile([128, S], f32, tag="a")
                        for k in range(DT):
                            for c in range(2):
                                cs = slice(c * 512, (c + 1) * 512)
                                mm(ps[:, cs], w1t[:, k, lm * 128:(lm + 1) * 128],
                                   bufC[:, k, cs], k == 0, k == DT - 1)
                        hb = hbufs[mi // 8]
                        nc.scalar.activation(hb[:, mi % 8, :], ps, AF.Gelu,
                                             bias=bias_t['ffn_b1'][:, ft:ft + 1],
                                             scale=1.0)
                w2a = load_w(w2_d[:, 2 * hf, :, :])
                w2b = load_w(w2_d[:, 2 * hf + 1, :, :])
                for m in range(DT):
                    ps = ps_a.tile([128, S], f32, tag="a")
                    for kq2, wt2 in enumerate((w2a, w2b)):
                        for k8 in range(DT):
                            k2 = kq2 * 8 + k8
                            for c in range(2):
                                cs = slice(c * 512, (c + 1) * 512)
                                mm(ps[:, cs], wt2[:, k8, m * 128:(m + 1) * 128],
                                   hbufs[k2 // 8][:, k2 % 8, cs],
                                   k2 == 0, k2 == 15)
                    if hf == 0:
                        nc.vector.scalar_tensor_tensor(
                            bufB[:, m, :], ps, 1.0, bufC[:, m, :],
                            ALU.mult, ALU.add)
                    else:
                        nc.vector.scalar_tensor_tensor(
                            bufB[:, m, :], ps,
                            bias_t['ffn_b2'][:, m:m + 1], bufB[:, m, :],
                            ALU.add, ALU.add)

            # P8: transpose outT (bufB, bf16) -> natural chunks, 8 stores
            for j in range(ST):
                och = chk.tile([128, D], f32, tag="chk", name="och")
                for i in range(DT):
                    tp = ps_r.tile([128, 512], f32, tag="r")
                    tpb = tp[:, 0:64].bitcast(bf16)
                    nc.tensor.transpose(
                        tpb, bufB[:, i, j * 128:(j + 1) * 128], ident_b)
                    nc.vector.tensor_copy(och[:, i * 128:(i + 1) * 128], tpb)
                nc.sync.dma_start(
                    out_d[j * 128:(j + 1) * 128, :].bitcast(f32r),
                    och.bitcast(f32r))

        if iters == 1:
            block_body()
        else:
            with tc.For_i(0, iters, 1):
                block_body()


_CACHE = {}


def _get_nc(iters=1):
    if iters not in _CACHE:
        _CACHE[iters] = _build(iters)
    return _CACHE[iters]


def _in_maps(inputs):
    shared = {}
    for n in B_NAMES + LN_NAMES + ['ffn_b1', 'ffn_b2']:
        shared[n] = np.ascontiguousarray(np.asarray(inputs[n], dtype=np.float32))
    for n in W_NAMES:
        w = np.asarray(inputs[n], dtype=np.float32).astype(ml_dtypes.bfloat16)
        shared[n] = np.ascontiguousarray(
            w.reshape(DT, 128, D).transpose(1, 0, 2))
    w1 = np.asarray(inputs['ffn_w1'], dtype=np.float32).astype(ml_dtypes.bfloat16)
    shared['ffn_w1'] = np.ascontiguousarray(
        w1.reshape(DT, 128, 4, D).transpose(1, 2, 0, 3))
    w2 = np.asarray(inputs['ffn_w2'], dtype=np.float32).astype(ml_dtypes.bfloat16)
    shared['ffn_w2'] = np.ascontiguousarray(
        w2.reshape(4, DT, 128, D).transpose(2, 0, 1, 3))
    for n in ('sa_bv', 'ca_bv'):
        shared[n] = shared[n].reshape(1, D)
    dec = np.asarray(inputs['decoder'], dtype=np.float32)
    enc = np.asarray(inputs['encoder'], dtype=np.float32)
    maps = []
    for b in range(B):
        m = dict(shared)
        m['decoder'] = np.ascontiguousarray(dec[b])
        m['encoder'] = np.ascontiguousarray(enc[b])
        maps.append(m)
    return maps


def kernel(**inputs):
    nc = _get_nc(1)
    res = run_bass_kernel_spmd(nc, _in_maps(inputs), core_ids=list(range(B)))
    return np.stack([res.results[b]['out'] for b in range(B)], axis=0)
